# revision 1
# baseline (speedup 1.0000x reference)
"""CrossModalAttention Trainium2 kernel (8 NeuronCores, tensor-parallel heads).

Strategy:
  - Head-parallel: 16 heads / 8 cores = 2 heads per core for both attention
    passes. Each core computes its 128-channel slice of Q/K/V for both
    modalities (QKV weights column-sliced per core, activations replicated).
  - All matmul operands are pre-transposed on the host so every device matmul
    contracts over the partition dim with no on-device layout fixups:
      QT/KT = W_slice-chunks @ X^T  -> [128ch, 8192tok] channel-major.
  - Attention computed as ST = scores^T [k, q]; exp(ST) feeds the A@V matmul
    directly as the moving operand. A ones-column appended to V yields the
    softmax denominators inside the same PSUM accumulation.
  - No max-subtraction in softmax: logits here are ~N(0,1); exp is safe.
  - fused = f_ta + f_tb written token-major per batch; AllToAll redistributes
    to token-parallel layout; LayerNorm (gamma/beta folded into W_proj on
    host) + projection run distributed over tokens.
  - Matmuls run in float32r (1 cyc/row at N=512 vs 4 for fp32); everything
    around them (softmax normalization, LayerNorm, PSUM) stays fp32.
"""

import numpy as np

import concourse.bacc as bacc
import concourse.mybir as mybir
import concourse.tile as tile
from concourse.bass_utils import run_bass_kernel_spmd

NC = 8            # cores
B = 4             # batch
N = 2048          # seq len
T = B * N         # total tokens = 8192
D = 1024          # model dim
H = 16            # heads
HPC = H // NC     # heads per core = 2
HD = D // H       # head dim = 64
S = D // NC       # channel slice per core = 128
SCALE = HD ** -0.5
KC = 16           # k chunks of 128 per batch
QC = 4            # q chunks of 512 per batch
DK = 8            # contraction chunks of 128 over D
EPS = 1e-5

F32 = mybir.dt.float32
F32R = mybir.dt.float32r
MM = F32R  # matmul operand dtype


def _build_nc(single=False):
    """single=True: 1-core variant for TimelineSim (collective replaced by a
    local DMA copy of the same buffers) — timing analysis only."""
    nc = bacc.Bacc("TRN2", target_bir_lowering=False, debug=False,
                   num_devices=(1 if single else NC))

    # ---- I/O ----
    xt_a = nc.dram_tensor("xt_a", [128, DK, T], MM, kind="ExternalInput")
    xt_b = nc.dram_tensor("xt_b", [128, DK, T], MM, kind="ExternalInput")
    wnames = ["wq_a", "wk_a", "wv_a", "wq_b", "wk_b", "wv_b"]
    w_in = {m: nc.dram_tensor(m, [128, DK, S], MM, kind="ExternalInput") for m in wnames}
    b_in = {m: nc.dram_tensor("b" + m[1:], [S, 1], F32, kind="ExternalInput") for m in wnames}
    wpt = nc.dram_tensor("wpt", [128, DK, D], MM, kind="ExternalInput")
    beff = nc.dram_tensor("beff", [1, D], MM, kind="ExternalInput")
    ident_r_in = nc.dram_tensor("ident_r", [128, 128], MM, kind="ExternalInput")
    ident_f_in = nc.dram_tensor("ident_f", [128, 128], F32, kind="ExternalInput")
    onescol_in = nc.dram_tensor("onescol", [128, KC, HPC, 1], MM, kind="ExternalInput")
    onesrow_in = nc.dram_tensor("onesrow", [1, 128], MM, kind="ExternalInput")
    out = nc.dram_tensor("out", [T // NC, D], F32, kind="ExternalOutput")

    with tile.TileContext(nc) as tc:
        with (
            tc.tile_pool(name="const", bufs=1) as constp,
            tc.tile_pool(name="sb", bufs=1) as sb,
            tc.tile_pool(name="ps", bufs=1, space="PSUM") as ps,
            tc.tile_pool(name="dram", bufs=1, space="DRAM") as dram,
        ):
            # ---- constants ----
            ident_r = constp.tile([128, 128], MM)
            nc.sync.dma_start(ident_r[:], ident_r_in[:])
            ident_f = constp.tile([128, 128], F32)
            nc.sync.dma_start(ident_f[:], ident_f_in[:])
            onesrow = constp.tile([1, 128], MM)
            nc.sync.dma_start(onesrow[:], onesrow_in[:])
            beff_sb = constp.tile([1, D], MM)
            nc.sync.dma_start(beff_sb[:], beff[:])

            # ---- weights resident in SBUF ----
            wsb = {}
            bsb = {}
            for m in wnames:
                w = sb.tile([128, DK, S], MM, name=f"w_{m}", tag=f"w_{m}")
                nc.sync.dma_start(w[:], w_in[m][:])
                wsb[m] = w
                bt = sb.tile([S, 1], F32, name=f"b_{m}", tag=f"b_{m}")
                nc.sync.dma_start(bt[:], b_in[m][:])
                bsb[m] = bt

            # ---- internal DRAM: channel-major projections [128, T] ----
            proj_dram = {}
            for nm in ["qta", "ktb", "vtb", "qtb", "kta", "vta"]:
                proj_dram[nm] = dram.tile([128, T], MM, name=f"d_{nm}", tag=f"d_{nm}")
            fused_d = [dram.tile([N, S], F32, name=f"fused{b}", tag=f"fused{b}") for b in range(B)]
            a2a_d = [dram.tile([N, S], F32, name=f"a2a{b}", tag=f"a2a{b}") for b in range(B)]

            # ================= Phase 1: QKV projections =================
            # dst = (W_slice @ X^T) + bias : [128 ch, T] channel-major
            plan = [
                (xt_a, [("wq_a", "qta"), ("wk_a", "kta"), ("wv_a", "vta")]),
                (xt_b, [("wk_b", "ktb"), ("wv_b", "vtb"), ("wq_b", "qtb")]),
            ]
            # Phase 1 is emitted batch-interleaved with phase 2 so the
            # scheduler round-robins PE between projections and attention
            # and ACT (exp) starts early.
            TW = 512

            def emit_proj_chunk(tch, sbuf_dst=None):
                for src, projs in plan:
                    xs = sb.tile([128, DK, TW], MM, name="xs", tag="big16", bufs=2)
                    nc.sync.dma_start(xs[:], src[:, :, tch * TW:(tch + 1) * TW])
                    for m, dst in projs:
                        pp = ps.tile([128, TW], F32, name="pp", tag="pp", bufs=1)
                        for k in range(DK):
                            nc.tensor.matmul(
                                pp[:], wsb[m][:, k, :], xs[:, k, :],
                                start=(k == 0), stop=(k == DK - 1),
                            )
                        if sbuf_dst is not None:
                            # write straight into the attention SBUF tile
                            # (skips the DRAM round trip on the critical path)
                            t, c0 = sbuf_dst[dst]
                            nc.vector.tensor_scalar_add(
                                t[:, c0 + tch * TW:c0 + (tch + 1) * TW],
                                pp[:], bsb[m][:, 0:1])
                        else:
                            ob = sb.tile([128, TW], MM, name="ob", tag="ob", bufs=3)
                            nc.vector.tensor_scalar_add(ob[:], pp[:], bsb[m][:, 0:1])
                            nc.sync.dma_start(
                                proj_dram[dst][:, tch * TW:(tch + 1) * TW], ob[:]
                            )

            # ================= Phase 2: attention =================
            # pass 0: f_ta = attn(Qa, Kb, Vb); pass 1: f_tb = attn(Qb, Ka, Va)
            osb_all = {b: {} for b in range(B)}

            def emit_attn_loads(b, pas):
                qt, kt, vt = (("qta", "ktb", "vtb") if pas == 0
                              else ("qtb", "kta", "vta"))
                qsb = sb.tile([128, N], MM, name="qsb", tag="qsb", bufs=2)
                nc.sync.dma_start(qsb[:], proj_dram[qt][:, b * N:(b + 1) * N])
                ksb = sb.tile([128, N], MM, name="ksb", tag="ksb", bufs=2)
                nc.sync.dma_start(ksb[:], proj_dram[kt][:, b * N:(b + 1) * N])
                vsb = sb.tile([128, N], MM, name="vsb", tag="vsb", bufs=2)
                nc.sync.dma_start(vsb[:], proj_dram[vt][:, b * N:(b + 1) * N])
                return qsb, ksb, vsb

            def emit_attn_pass(b, pas, loads):
                osb = osb_all[b]
                qsb, ksb, vsb = loads

                # V^T [128ch, N] -> token-major V_aug tiles [128k, 65]
                # (col 64 = 1.0 for softmax denominators)
                vaug = sb.tile([128, KC, HPC, 65], MM, name="vaug", tag="vaug", bufs=2)
                nc.sync.dma_start(vaug[:, :, :, 64:65], onescol_in[:])
                for kc in range(KC):
                    tp = ps.tile([128, 128], MM, name="tp", tag="misc", bufs=1)
                    nc.tensor.transpose(tp[:], vsb[:, kc * 128:(kc + 1) * 128], ident_r[:])
                    for h in range(HPC):
                        nc.vector.tensor_copy(vaug[:, kc, h, 0:64], tp[:, h * HD:(h + 1) * HD])

                for h in range(HPC):
                    hsl = slice(h * HD, (h + 1) * HD)
                    for qc in range(QC):
                        qsl = slice(qc * 512, (qc + 1) * 512)
                        av = ps.tile([65, 512], F32, name="av", tag="av", bufs=2)
                        for kc2 in range(KC // 2):
                            # two score chunks into one 2-bank PSUM tile,
                            # one wide exp (halves ACT per-op overhead)
                            sp = ps.tile([128, 1024], F32, name="sp", tag="sp", bufs=2)
                            ex = sb.tile([128, 1024], MM, name="ex", tag="t512", bufs=4)
                            for j in range(2):
                                kc = kc2 * 2 + j
                                nc.tensor.matmul(
                                    sp[:, j * 512:(j + 1) * 512],
                                    ksb[hsl, kc * 128:(kc + 1) * 128],
                                    qsb[hsl, qsl],
                                    start=True, stop=True,
                                )
                            nc.scalar.activation(
                                ex[:], sp[:], mybir.ActivationFunctionType.Exp,
                                scale=SCALE,
                            )
                            for j in range(2):
                                kc = kc2 * 2 + j
                                nc.tensor.matmul(
                                    av[:], vaug[:, kc, h, :],
                                    ex[:, j * 512:(j + 1) * 512],
                                    start=(kc == 0), stop=(kc == KC - 1),
                                )
                        # drain PSUM with one copy so the av slot frees
                        # fast, then normalize from SBUF
                        avs = sb.tile([65, 512], F32, name="avs", tag="avs", bufs=3)
                        nc.vector.tensor_copy(avs[:], av[:])
                        rec = sb.tile([1, 512], F32, name="rec", tag="rec", bufs=2)
                        nc.vector.reciprocal(rec[:], avs[64:65, :])
                        rbs = sb.tile([64, 512], F32, name="rbs", tag="rbs", bufs=2)
                        nc.gpsimd.partition_broadcast(rbs[:], rec[:])
                        if pas == 0:
                            o = sb.tile([64, 512], F32, name="o",
                                        tag=f"osb{h}{qc}", bufs=1)
                            nc.vector.tensor_mul(o[:], avs[0:64, :], rbs[:])
                            osb[(h, qc)] = o
                        else:
                            fs = sb.tile([64, 512], F32, name="fs", tag="fs", bufs=2)
                            nc.vector.tensor_mul(fs[:], avs[0:64, :], rbs[:])
                            nc.vector.tensor_add(fs[:], fs[:], osb[(h, qc)][:])
                            # token-major transpose + store fused slice
                            for qi in range(4):
                                ftp = ps.tile([128, 64], F32, name="ftp",
                                              tag="misc", bufs=1)
                                nc.tensor.transpose(
                                    ftp[:], fs[:, qi * 128:(qi + 1) * 128],
                                    ident_f[0:64, 0:64],
                                )
                                fts = sb.tile([128, 64], F32, name="fts",
                                              tag="fts", bufs=3)
                                nc.vector.tensor_copy(fts[:], ftp[:])
                                r0 = qc * 512 + qi * 128
                                nc.sync.dma_start(
                                    fused_d[b][r0:r0 + 128, h * HD:(h + 1) * HD],
                                    fts[:],
                                )

            def emit_a2a(b):
                if single:
                    nc.sync.dma_start(a2a_d[b][:], fused_d[b][:])
                else:
                    nc.gpsimd.collective_compute(
                        "AllToAll", mybir.AluOpType.bypass,
                        replica_groups=[list(range(NC))],
                        ins=[fused_d[b].opt()], outs=[a2a_d[b].opt()],
                    )

            # interleave: proj chunks for batch b, then that batch's Q/K/V
            # loads (ahead of the next proj burst in the DMA queues), with
            # proj bursts emitted after the attention pass they overlap
            q0 = sb.tile([128, N], MM, name="q0", tag="qsb", bufs=2)
            k0 = sb.tile([128, N], MM, name="k0", tag="ksb", bufs=2)
            v0 = sb.tile([128, N], MM, name="v0", tag="vsb", bufs=2)
            q1 = sb.tile([128, N], MM, name="q1", tag="qsb", bufs=2)
            k1 = sb.tile([128, N], MM, name="k1", tag="ksb", bufs=2)
            v1 = sb.tile([128, N], MM, name="v1", tag="vsb", bufs=2)
            b0_dst = {"qta": (q0, 0), "ktb": (k0, 0), "vtb": (v0, 0),
                      "qtb": (q1, 0), "kta": (k1, 0), "vta": (v1, 0)}
            for tch in range(4):
                emit_proj_chunk(tch, sbuf_dst=b0_dst)   # batch 0 -> SBUF
            l00 = (q0, k0, v0)
            l01 = (q1, k1, v1)
            emit_attn_pass(0, 0, l00)
            for tch in range(4, 8):
                emit_proj_chunk(tch)          # batch 1 columns
            emit_attn_pass(0, 1, l01)
            emit_a2a(0)
            l10 = emit_attn_loads(1, 0)
            l11 = emit_attn_loads(1, 1)
            emit_attn_pass(1, 0, l10)
            for tch in range(8, 12):
                emit_proj_chunk(tch)          # batch 2 columns
            emit_attn_pass(1, 1, l11)
            emit_a2a(1)
            l20 = emit_attn_loads(2, 0)
            l21 = emit_attn_loads(2, 1)
            emit_attn_pass(2, 0, l20)
            emit_attn_pass(2, 1, l21)
            for tch in range(12, 16):
                emit_proj_chunk(tch)          # batch 3 columns
            emit_a2a(2)
            l30 = emit_attn_loads(3, 0)
            l31 = emit_attn_loads(3, 1)
            emit_attn_pass(3, 0, l30)
            emit_attn_pass(3, 1, l31)
            emit_a2a(3)

            # ================= Phase 3: LayerNorm + projection =================
            # a2a_d[b] rows [i*256:(i+1)*256] = channel block i of this core's
            # 256 tokens of batch b.
            TB = N // NC  # 256 tokens per core per batch
            # wpt loaded now (reuses the big16 slots freed after phase 1)
            wph = []
            for oc in range(2):
                wp = sb.tile([128, DK, 512], MM, name=f"wph{oc}", tag="big16", bufs=2)
                nc.sync.dma_start(wp[:], wpt[:, :, oc * 512:(oc + 1) * 512])
                wph.append(wp)
            for b in range(B):
                av3 = a2a_d[b][:].rearrange("(i r) c -> r i c", i=NC)  # [256, 8, 128]
                for tt in range(TB // 128):
                    x = sb.tile([128, NC, S], F32, name="x", tag="x", bufs=2)
                    nc.sync.dma_start(x[:], av3[tt * 128:(tt + 1) * 128])
                    xf = x[:].rearrange("p i c -> p (i c)")  # [128, 1024]
                    ssum = sb.tile([128, 1], F32, name="ssum", tag="stat", bufs=4)
                    nc.vector.reduce_sum(ssum[:], xf, axis=mybir.AxisListType.X)
                    mu = sb.tile([128, 1], F32, name="mu", tag="stat", bufs=4)
                    nc.vector.tensor_scalar_mul(mu[:], ssum[:], 1.0 / D)
                    xc = sb.tile([128, D], F32, name="xc", tag="xc", bufs=2)
                    nc.vector.tensor_scalar(xc[:], xf, mu[:, 0:1], None,
                                            op0=mybir.AluOpType.subtract)
                    sq = sb.tile([128, D], F32, name="sq", tag="x", bufs=2)
                    nc.vector.tensor_mul(sq[:], xc[:], xc[:])
                    vs = sb.tile([128, 1], F32, name="vs", tag="stat", bufs=4)
                    nc.vector.reduce_sum(vs[:], sq[:], axis=mybir.AxisListType.X)
                    var = sb.tile([128, 1], F32, name="var", tag="stat", bufs=4)
                    nc.vector.tensor_scalar(var[:], vs[:], 1.0 / D, EPS,
                                            op0=mybir.AluOpType.mult,
                                            op1=mybir.AluOpType.add)
                    inv = sb.tile([128, 1], F32, name="inv", tag="stat", bufs=4)
                    nc.vector.reciprocal(inv[:], var[:])
                    rstd = sb.tile([128, 1], F32, name="rstd", tag="stat", bufs=4)
                    nc.scalar.sqrt(rstd[:], inv[:])
                    xn = sb.tile([128, D], MM, name="xn", tag="xc", bufs=2)
                    nc.vector.tensor_scalar_mul(xn[:], xc[:], rstd[:, 0:1])
                    # transpose to [d, t] chunks
                    fnT = sb.tile([128, DK, 128], MM, name="fnT", tag="fnT", bufs=2)
                    for k in range(DK):
                        tp3 = ps.tile([128, 128], MM, name="tp3", tag="misc", bufs=1)
                        nc.tensor.transpose(tp3[:], xn[:, k * 128:(k + 1) * 128], ident_r[:])
                        nc.vector.tensor_copy(fnT[:, k, :], tp3[:])
                    # out tile rows
                    r0 = b * TB + tt * 128
                    for oc in range(2):
                        osl = slice(oc * 512, (oc + 1) * 512)
                        pp3 = ps.tile([128, 512], F32, name="pp3", tag="pp", bufs=1)
                        for k in range(DK):
                            nc.tensor.matmul(pp3[:], fnT[:, k, :],
                                             wph[oc][:, k, :],
                                             start=(k == 0), stop=False)
                        nc.tensor.matmul(pp3[:], onesrow[0:1, :],
                                         beff_sb[0:1, osl],
                                         start=False, stop=True)
                        ou = sb.tile([128, 512], F32, name="ou", tag="t512", bufs=4)
                        nc.vector.tensor_copy(ou[:], pp3[:])
                        nc.sync.dma_start(out[r0:r0 + 128, osl], ou[:])

    nc.compile()
    return nc


_NC_CACHE = None


def _get_nc():
    global _NC_CACHE
    if _NC_CACHE is None:
        _NC_CACHE = _build_nc()
    return _NC_CACHE


def _prep_w(w):
    """[out_rows, D] weight slice -> transposed chunked [128, DK, out] f32."""
    wt = np.ascontiguousarray(w.T)          # [D, out]
    o = wt.shape[1]
    return np.ascontiguousarray(
        wt.reshape(DK, 128, o).transpose(1, 0, 2), dtype=np.float32
    )


def _make_in_maps(inputs):
    f_a = np.asarray(inputs["f_a"], np.float32).reshape(T, D)
    f_b = np.asarray(inputs["f_b"], np.float32).reshape(T, D)
    gamma = np.asarray(inputs["ln_gamma"], np.float32)
    beta = np.asarray(inputs["ln_beta"], np.float32)
    W_proj = np.asarray(inputs["W_proj"], np.float32)
    b_proj = np.asarray(inputs["b_proj"], np.float32)

    xt_a = _prep_w(f_a)  # [128, DK, T]
    xt_b = _prep_w(f_b)

    # fold LN gamma/beta into projection: y = (ln01(x)*g+bt) @ Wp^T + bp
    #   = ln01(x) @ (Wp*g)^T + (bt @ Wp^T + bp)
    w_eff = W_proj * gamma[None, :]
    b_eff = (b_proj + beta @ W_proj.T).astype(np.float32)
    wpt = _prep_w(w_eff)  # [128, DK, D]
    ident = np.eye(128, dtype=np.float32)

    wmap = {"wq_a": "W_q_a", "wk_a": "W_k_a", "wv_a": "W_v_a",
            "wq_b": "W_q_b", "wk_b": "W_k_b", "wv_b": "W_v_b"}

    in_maps = []
    for c in range(NC):
        sl = slice(c * S, (c + 1) * S)
        m = {"xt_a": xt_a, "xt_b": xt_b, "wpt": wpt,
             "beff": b_eff.reshape(1, D),
             "ident_r": ident, "ident_f": ident,
             "onescol": np.ones((128, KC, HPC, 1), np.float32),
             "onesrow": np.ones((1, 128), np.float32)}
        for dev_name, inp_name in wmap.items():
            m[dev_name] = _prep_w(np.asarray(inputs[inp_name], np.float32)[sl, :])
            m["b" + dev_name[1:]] = np.ascontiguousarray(
                np.asarray(inputs["b" + inp_name[1:]], np.float32)[sl].reshape(S, 1)
            )
        in_maps.append(m)
    return in_maps


def _assemble(outs):
    """outs: list of per-core 'out' arrays [T//NC, D] -> [B, N, D]."""
    TB = N // NC
    full = np.empty((T, D), np.float32)
    for c in range(NC):
        oc = outs[c].reshape(B, TB, D)
        for b in range(B):
            full[b * N + c * TB: b * N + (c + 1) * TB] = oc[b]
    return full.reshape(B, N, D)


def kernel(**inputs):
    in_maps = _make_in_maps(inputs)
    nc = _get_nc()
    res = run_bass_kernel_spmd(nc, in_maps, list(range(NC)))
    return _assemble([res.results[c]["out"] for c in range(NC)])

